# revision 4
# baseline (speedup 1.0000x reference)
"""Linearized-Hopf CSD covariance on 8 Trainium2 NeuronCores.

Math: for each frequency f, M(f) = d(f)*I - C(f) with scalar diagonal
d = -a + i*(om - omega0) and C = W .* exp(-i*om*delay), W row-normalized
(row sums == g).  H = M^-1 and the output is
  eeg_cov = cy0*df*std^2 * sum_f Re((lm H)(lm H)^H).
Since ||C/d||_inf = g/|d| < 1, G = lm H is computed by the Horner/
Neumann recurrence  Z <- lm + Z @ Ct  (Ct = C/d, G = Z_m / d), with a
per-frequency degree m chosen from the exact contraction factor
rho = g/|d| for truncation error < tol.  Frequencies are sharded
strided across the 8 cores (core c gets indices c, c+8, ...) so each
core receives the same mix of easy / near-resonance frequencies.

Device mapping (per core):
- Complex products use rhs concatenation: cat1 = [Ctr | Cti] and
  cat2 = [-Cti | Ctr] (N=400), so one stationary pass per component.
- Off-resonance frequencies run in bf16, PAIRED two-per-128-partitions
  (PSUM col-groups 0:64 / 64:128) so matmuls pack into disjoint PE
  column groups and the transposes / combines / copies are shared.
  Their absolute contribution to the summed covariance is 100-1000x
  below the resonant band, so bf16 error is negligible.
- The near-resonance frequencies (deepest recurrences) run solo in
  float32r (full fp32 storage, fast PE mode).
- The output scale s_f = sqrt(std^2*df*cy0)/|d_f| is folded on the
  host into the FINAL round's cat matrices and lm-constant, so the
  channel covariance accumulates in PSUM with no extra scaling ops:
  paired class into a [128,128] block-diagonal accumulator, solo class
  into a [64,64] accumulator; the host sums the halves + 8 cores.
"""

import sys

sys.path.insert(0, "/opt/trn_rl_repo")

import numpy as np
import ml_dtypes

N = 200
O = 64
F = 512
NCORES = 8
FPC = F // NCORES
P_HI = 128
P_LO = N - P_HI
NCAT = 2 * N          # 400
TOL = 1e-3
NSOLO_F32 = 2         # highest-degree positions run solo in f32r

_compiled = {}
PROFILE = False
LAST = None


def _plan(degrees):
    """Group the 64 per-core positions into equal-degree bf16 pairs and
    solo frequencies. Returns (pairs, solo_bf, solo_f32) of position ids,
    highest degree first."""
    order = sorted(range(FPC), key=lambda j: -degrees[j])
    solo_f32 = order[:NSOLO_F32]
    rest = order[NSOLO_F32:]
    pairs, solo_bf = [], []
    by_deg = {}
    for j in rest:
        by_deg.setdefault(degrees[j], []).append(j)
    for deg, js in sorted(by_deg.items(), key=lambda kv: -kv[0]):
        while len(js) >= 2:
            pairs.append((js.pop(0), js.pop(0)))
        if js:
            solo_bf.append(js[0])
    return pairs, solo_bf, solo_f32


def _build_program(degrees):
    import concourse.bacc as bacc
    import concourse.mybir as mybir
    from concourse.tile import TileContext

    f32 = mybir.dt.float32
    f32r = mybir.dt.float32r
    bf16 = mybir.dt.bfloat16

    pairs, solo_bf, solo_f32 = _plan(degrees)

    # DMA slot layout for the bf16 cat tensor: per paired/solo-bf freq,
    # rounds 1..m; round 1 needs only cat1 (Z0 is real); the final round
    # uses the s-scaled cats. Slot key -> index, sized at the end.
    slot_bf = {}   # (pos, round, cat) -> slot index
    for j in [x for p in pairs for x in p] + solo_bf:
        m = degrees[j]
        for r in range(1, m + 1):
            slot_bf[(j, r, 1)] = len(slot_bf)
            if r > 1:
                slot_bf[(j, r, 2)] = len(slot_bf)
    # f32 cats for solo freqs: unscaled cat1/cat2 + scaled cat1/cat2
    slot_fr = {}
    for j in solo_f32:
        m = degrees[j]
        slot_fr[(j, "u", 1)] = len(slot_fr)
        if m > 1:
            slot_fr[(j, "u", 2)] = len(slot_fr)
        slot_fr[(j, "s", 1)] = len(slot_fr)
        if m > 1:
            slot_fr[(j, "s", 2)] = len(slot_fr)

    npair = len(pairs)
    nsbf = len(solo_bf)
    nsfr = len(solo_f32)

    nc = bacc.Bacc(None, target_bir_lowering=False, debug=False)

    catb_d = nc.dram_tensor("catb", [max(len(slot_bf), 1), N, NCAT], bf16,
                            kind="ExternalInput")
    catf_d = nc.dram_tensor("catf", [max(len(slot_fr), 1), N, NCAT], f32r,
                            kind="ExternalInput")
    lmTb_d = nc.dram_tensor("lmTb", [N, O], bf16, kind="ExternalInput")
    lmTf_d = nc.dram_tensor("lmTf", [N, O], f32r, kind="ExternalInput")
    lmp_d = nc.dram_tensor("lmp", [128, N], f32, kind="ExternalInput")
    slmp_d = nc.dram_tensor("slmp", [max(npair, 1), 128, N], f32,
                            kind="ExternalInput")
    slmb_d = nc.dram_tensor("slmb", [max(nsbf, 1), O, N], f32,
                            kind="ExternalInput")
    slmf_d = nc.dram_tensor("slmf", [max(nsfr, 1), O, N], f32,
                            kind="ExternalInput")
    eyeb_d = nc.dram_tensor("eyeb", [128, 128], bf16, kind="ExternalInput")
    eyef_d = nc.dram_tensor("eyef", [O, O], f32, kind="ExternalInput")
    out_d = nc.dram_tensor("out_e", [O, 2 * O], f32, kind="ExternalOutput")

    with TileContext(nc) as tc:
        with tc.tile_pool(name="consts", bufs=1) as cpool, \
             tc.tile_pool(name="cmatb", bufs=6) as cmatb, \
             tc.tile_pool(name="cmatf", bufs=1) as cmatf, \
             tc.tile_pool(name="zbuf", bufs=3) as zbuf, \
             tc.tile_pool(name="stat", bufs=3) as stat, \
             tc.tile_pool(name="slmbuf", bufs=3) as slmbuf, \
             tc.tile_pool(name="pmm", bufs=3, space="PSUM") as pmm, \
             tc.tile_pool(name="pzt", bufs=3, space="PSUM") as pzt, \
             tc.tile_pool(name="pe1", bufs=1, space="PSUM") as pe1, \
             tc.tile_pool(name="pe2", bufs=1, space="PSUM") as pe2:

            lmTb_hi = cpool.tile([P_HI, O], bf16)
            nc.sync.dma_start(lmTb_hi[:], lmTb_d[0:P_HI, :])
            lmTb_lo = cpool.tile([P_LO, O], bf16)
            nc.sync.dma_start(lmTb_lo[:], lmTb_d[P_HI:N, :])
            lmTf_hi = cpool.tile([P_HI, O], f32r)
            nc.sync.dma_start(lmTf_hi[:], lmTf_d[0:P_HI, :])
            lmTf_lo = cpool.tile([P_LO, O], f32r)
            nc.sync.dma_start(lmTf_lo[:], lmTf_d[P_HI:N, :])
            lmp_sb = cpool.tile([128, N], f32)
            nc.sync.dma_start(lmp_sb[:], lmp_d[:])
            eyeb_sb = cpool.tile([128, 128], bf16)
            nc.sync.dma_start(eyeb_sb[:], eyeb_d[:])
            eyef_sb = cpool.tile([O, O], f32)
            nc.sync.dma_start(eyef_sb[:], eyef_d[:])

            pE1 = pe1.tile([128, 128], f32)   # paired-class accumulator
            pE2 = pe2.tile([O, O], f32)       # solo-f32r accumulator
            e1_state = [True]
            e2_state = [True]
            # count total E-matmuls to set stop on the last one
            ne1_total = (npair + nsbf) * 4
            ne2_total = nsfr * 4
            ne1 = [0]
            ne2 = [0]

            def dma_cat_bf(pos, r, cat):
                s = slot_bf[(pos, r, cat)]
                h = cmatb.tile([P_HI, NCAT], bf16, tag=f"c{cat}h", name=f"cb{cat}h")
                nc.sync.dma_start(h[:], catb_d[s, 0:P_HI, :])
                lo = cmatb.tile([P_LO, NCAT], bf16, tag=f"c{cat}l", name=f"cb{cat}l")
                nc.sync.dma_start(lo[:], catb_d[s, P_HI:N, :])
                return h, lo

            def dma_cat_fr(pos, sc, cat):
                s = slot_fr[(pos, sc, cat)]
                h = cmatf.tile([P_HI, NCAT], f32r, tag=f"f{pos}{sc}{cat}h",
                               name=f"cf{cat}h")
                nc.sync.dma_start(h[:], catf_d[s, 0:P_HI, :])
                lo = cmatf.tile([P_LO, NCAT], f32r, tag=f"f{pos}{sc}{cat}l",
                                name=f"cf{cat}l")
                nc.sync.dma_start(lo[:], catf_d[s, P_HI:N, :])
                return h, lo

            def emit_pair(pi, jA, jB):
                m = degrees[jA]
                ztc = None
                for r in range(1, m + 1):
                    final = r == m
                    p = pmm.tile([128, NCAT], f32, tag="p", name="p")
                    for half, j in ((0, jA), (1, jB)):
                        po = p[64 * half:64 * half + 64, :]
                        c1h, c1l = dma_cat_bf(j, r, 1)
                        if r == 1:
                            nc.tensor.matmul(po, lmTb_hi[:], c1h[:],
                                             start=True, stop=False,
                                             skip_group_check=True)
                            nc.tensor.matmul(po, lmTb_lo[:], c1l[:],
                                             start=False, stop=True,
                                             skip_group_check=True)
                        else:
                            c2h, c2l = dma_cat_bf(j, r, 2)
                            zr_hi = ztc[0][:, 64 * half:64 * half + 64]
                            zr_lo = ztc[1][0:P_LO, 64 * half:64 * half + 64]
                            zi_hi = ztc[2][:, 64 * half:64 * half + 64]
                            zi_lo = ztc[3][0:P_LO, 64 * half:64 * half + 64]
                            nc.tensor.matmul(po, zr_hi, c1h[:], start=True,
                                             stop=False, skip_group_check=True)
                            nc.tensor.matmul(po, zr_lo, c1l[:], start=False,
                                             stop=False, skip_group_check=True)
                            nc.tensor.matmul(po, zi_hi, c2h[:], start=False,
                                             stop=False, skip_group_check=True)
                            nc.tensor.matmul(po, zi_lo, c2l[:], start=False,
                                             stop=True, skip_group_check=True)
                    # combine
                    if final:
                        sl = slmbuf.tile([128, N], f32, tag="slp", name="slp")
                        nc.sync.dma_start(sl[:], slmp_d[pi, :, :])
                    else:
                        sl = lmp_sb
                    zr = zbuf.tile([128, N], bf16, tag="zr", name="zr")
                    nc.vector.tensor_add(zr[:], p[:, 0:N], sl[:])
                    zi = zbuf.tile([128, N], bf16, tag="zi", name="zi")
                    nc.scalar.copy(zi[:], p[:, N:NCAT])
                    # transposes: one PSUM bank, bf16 [128, 512]
                    zt = pzt.tile([128, 512], bf16, tag="zt", name="zt")
                    nc.tensor.transpose(zt[0:P_HI, 0:128], zr[:, 0:P_HI],
                                        eyeb_sb[:])
                    nc.tensor.transpose(zt[0:P_LO, 128:256], zr[:, P_HI:N],
                                        eyeb_sb[:])
                    nc.tensor.transpose(zt[0:P_HI, 256:384], zi[:, 0:P_HI],
                                        eyeb_sb[:])
                    nc.tensor.transpose(zt[0:P_LO, 384:512], zi[:, P_HI:N],
                                        eyeb_sb[:])
                    trh = stat.tile([P_HI, 128], bf16, tag="trh", name="trh")
                    nc.scalar.copy(trh[:], zt[0:P_HI, 0:128])
                    trl = stat.tile([P_LO, 128], bf16, tag="trl", name="trl")
                    nc.vector.tensor_copy(trl[:], zt[0:P_LO, 128:256])
                    tih = stat.tile([P_HI, 128], bf16, tag="tih", name="tih")
                    nc.scalar.copy(tih[:], zt[0:P_HI, 256:384])
                    til = stat.tile([P_LO, 128], bf16, tag="til", name="til")
                    nc.vector.tensor_copy(til[:], zt[0:P_LO, 384:512])
                    ztc = (trh, trl, tih, til)
                # E accumulation: block-diagonal [128,128] += Zt^T Zt
                for t in ztc:
                    kk = t.shape[0] if hasattr(t, "shape") else None
                    ne1[0] += 1
                    nc.tensor.matmul(pE1[:], t[:], t[:],
                                     start=e1_state[0],
                                     stop=ne1[0] == ne1_total,
                                     skip_group_check=True)
                    e1_state[0] = False

            def emit_solo_bf(si, j):
                # unpaired bf16 frequency: same as pair but half-width
                m = degrees[j]
                ztc = None
                for r in range(1, m + 1):
                    final = r == m
                    p = pmm.tile([128, NCAT], f32, tag="p", name="p")
                    po = p[0:64, :]
                    c1h, c1l = dma_cat_bf(j, r, 1)
                    if r == 1:
                        nc.tensor.matmul(po, lmTb_hi[:], c1h[:], start=True,
                                         stop=False, skip_group_check=True)
                        nc.tensor.matmul(po, lmTb_lo[:], c1l[:], start=False,
                                         stop=True, skip_group_check=True)
                    else:
                        c2h, c2l = dma_cat_bf(j, r, 2)
                        nc.tensor.matmul(po, ztc[0][:, 0:64], c1h[:],
                                         start=True, stop=False,
                                         skip_group_check=True)
                        nc.tensor.matmul(po, ztc[1][0:P_LO, 0:64], c1l[:],
                                         start=False, stop=False,
                                         skip_group_check=True)
                        nc.tensor.matmul(po, ztc[2][:, 0:64], c2h[:],
                                         start=False, stop=False,
                                         skip_group_check=True)
                        nc.tensor.matmul(po, ztc[3][0:P_LO, 0:64], c2l[:],
                                         start=False, stop=True,
                                         skip_group_check=True)
                    if final:
                        sl = slmbuf.tile([O, N], f32, tag="slb", name="slb")
                        nc.sync.dma_start(sl[:], slmb_d[si, :, :])
                        slv = sl[:]
                    else:
                        slv = lmp_sb[0:64, :]
                    zr = zbuf.tile([128, N], bf16, tag="zr", name="zr")
                    nc.vector.tensor_add(zr[0:64, :], p[0:64, 0:N], slv)
                    zi = zbuf.tile([128, N], bf16, tag="zi", name="zi")
                    nc.scalar.copy(zi[0:64, :], p[0:64, N:NCAT])
                    zt = pzt.tile([128, 512], bf16, tag="zt", name="zt")
                    nc.tensor.transpose(zt[0:P_HI, 0:64], zr[0:64, 0:P_HI],
                                        eyeb_sb[0:64, 0:64])
                    nc.tensor.transpose(zt[0:P_LO, 128:192], zr[0:64, P_HI:N],
                                        eyeb_sb[0:64, 0:64])
                    nc.tensor.transpose(zt[0:P_HI, 256:320], zi[0:64, 0:P_HI],
                                        eyeb_sb[0:64, 0:64])
                    nc.tensor.transpose(zt[0:P_LO, 384:448], zi[0:64, P_HI:N],
                                        eyeb_sb[0:64, 0:64])
                    trh = stat.tile([P_HI, 128], bf16, tag="trh", name="trh")
                    nc.scalar.copy(trh[:, 0:64], zt[0:P_HI, 0:64])
                    trl = stat.tile([P_LO, 128], bf16, tag="trl", name="trl")
                    nc.vector.tensor_copy(trl[:, 0:64], zt[0:P_LO, 128:192])
                    tih = stat.tile([P_HI, 128], bf16, tag="tih", name="tih")
                    nc.scalar.copy(tih[:, 0:64], zt[0:P_HI, 256:320])
                    til = stat.tile([P_LO, 128], bf16, tag="til", name="til")
                    nc.vector.tensor_copy(til[:, 0:64], zt[0:P_LO, 384:448])
                    ztc = (trh, trl, tih, til)
                for t in ztc:
                    ne1[0] += 1
                    nc.tensor.matmul(pE1[0:64, 0:64], t[:, 0:64], t[:, 0:64],
                                     start=e1_state[0],
                                     stop=ne1[0] == ne1_total,
                                     skip_group_check=True)
                    e1_state[0] = False

            def emit_solo_fr(si, j):
                m = degrees[j]
                cu = None
                if m > 1:
                    cu = (dma_cat_fr(j, "u", 1), dma_cat_fr(j, "u", 2))
                cs1 = dma_cat_fr(j, "s", 1)
                cs2 = dma_cat_fr(j, "s", 2) if m > 1 else None
                ztc = None
                for r in range(1, m + 1):
                    final = r == m
                    if final:
                        c1h, c1l = cs1
                        c2hl = cs2
                    else:
                        c1h, c1l = cu[0]
                        c2hl = cu[1]
                    p = pmm.tile([128, NCAT], f32, tag="p", name="p")
                    po = p[0:64, :]
                    if r == 1:
                        nc.tensor.matmul(po, lmTf_hi[:], c1h[:], start=True,
                                         stop=False, skip_group_check=True)
                        nc.tensor.matmul(po, lmTf_lo[:], c1l[:], start=False,
                                         stop=True, skip_group_check=True)
                    else:
                        c2h, c2l = c2hl
                        nc.tensor.matmul(po, ztc[0][:], c1h[:], start=True,
                                         stop=False, skip_group_check=True)
                        nc.tensor.matmul(po, ztc[1][:], c1l[:], start=False,
                                         stop=False, skip_group_check=True)
                        nc.tensor.matmul(po, ztc[2][:], c2h[:], start=False,
                                         stop=False, skip_group_check=True)
                        nc.tensor.matmul(po, ztc[3][:], c2l[:], start=False,
                                         stop=True, skip_group_check=True)
                    if final:
                        sl = slmbuf.tile([O, N], f32, tag="slf", name="slf")
                        nc.sync.dma_start(sl[:], slmf_d[si, :, :])
                        slv = sl[:]
                    else:
                        slv = lmp_sb[0:64, :]
                    zr = zbuf.tile([O, N], f32, tag="zrf", name="zrf")
                    nc.vector.tensor_add(zr[:], p[0:64, 0:N], slv)
                    zi = zbuf.tile([O, N], f32, tag="zif", name="zif")
                    nc.scalar.copy(zi[:], p[0:64, N:NCAT])
                    zt = pzt.tile([128, 256], f32, tag="zt", name="ztf")
                    nc.tensor.transpose(zt[0:P_HI, 0:O], zr[:, 0:P_HI],
                                        eyef_sb[:])
                    nc.tensor.transpose(zt[0:P_LO, O:2 * O], zr[:, P_HI:N],
                                        eyef_sb[:])
                    nc.tensor.transpose(zt[0:P_HI, 2 * O:3 * O], zi[:, 0:P_HI],
                                        eyef_sb[:])
                    nc.tensor.transpose(zt[0:P_LO, 3 * O:4 * O], zi[:, P_HI:N],
                                        eyef_sb[:])
                    trh = stat.tile([P_HI, O], f32r, tag="ftrh", name="ftrh")
                    nc.scalar.copy(trh[:], zt[0:P_HI, 0:O])
                    trl = stat.tile([P_LO, O], f32r, tag="ftrl", name="ftrl")
                    nc.vector.tensor_copy(trl[:], zt[0:P_LO, O:2 * O])
                    tih = stat.tile([P_HI, O], f32r, tag="ftih", name="ftih")
                    nc.scalar.copy(tih[:], zt[0:P_HI, 2 * O:3 * O])
                    til = stat.tile([P_LO, O], f32r, tag="ftil", name="ftil")
                    nc.vector.tensor_copy(til[:], zt[0:P_LO, 3 * O:4 * O])
                    ztc = (trh, trl, tih, til)
                for t in ztc:
                    ne2[0] += 1
                    nc.tensor.matmul(pE2[:], t[:], t[:],
                                     start=e2_state[0],
                                     stop=ne2[0] == ne2_total,
                                     skip_group_check=True)
                    e2_state[0] = False

            # emit deepest chains first so they overlap everything else
            for si, j in enumerate(solo_f32):
                emit_solo_fr(si, j)
            for si, j in enumerate(solo_bf):
                emit_solo_bf(si, j)
            for pi, (jA, jB) in enumerate(pairs):
                emit_pair(pi, jA, jB)

            e_sb = cpool.tile([O, 2 * O], f32)
            nc.scalar.copy(e_sb[:, O:2 * O], pE1[O:128, O:128])
            nc.vector.tensor_add(e_sb[:, 0:O], pE1[0:O, 0:O],
                                 e_sb[:, O:2 * O])
            nc.scalar.copy(e_sb[:, O:2 * O], pE2[:])
            nc.sync.dma_start(out_d[:], e_sb[:])

    nc.compile()
    return nc, pairs, solo_bf, solo_f32, slot_bf, slot_fr


def kernel(sc, dist, freqs, lm, wll, a, omega, g, std_in, v_d, cy0):
    from concourse.bass_utils import run_bass_kernel_spmd

    sc = np.asarray(sc, np.float64)
    dist = np.asarray(dist, np.float64)
    freqs32 = np.asarray(freqs, np.float32)
    freqs = freqs32.astype(np.float64)
    lm = np.asarray(lm, np.float32)
    wll = np.asarray(wll, np.float64)
    a = float(a); omega = float(omega); g = float(g)
    std_in = float(std_in); v_d = float(v_d); cy0 = float(cy0)

    w = np.exp(wll) * sc
    w_n = g * w / (w.sum(axis=1, keepdims=True) + 1e-8)
    delay = dist / v_d
    om = 2.0 * np.pi * freqs
    d = -a + 1j * (om - omega)
    df = float(freqs32[1] - freqs32[0])

    rs = w_n.sum(axis=1).max()
    rho = rs / np.abs(d)
    with np.errstate(divide="ignore"):
        mf = np.ceil(np.log(TOL * (1.0 - rho)) / np.log(rho)) - 1.0
    mf = np.clip(np.nan_to_num(mf, nan=1.0), 1, 60).astype(int)

    degrees = tuple(
        int(max(mf[j * NCORES + c] for c in range(NCORES))) for j in range(FPC)
    )
    if degrees not in _compiled:
        _compiled[degrees] = _build_program(degrees)
    nc, pairs, solo_bf, solo_f32, slot_bf, slot_fr = _compiled[degrees]

    lmT = np.ascontiguousarray(lm.T)
    s_all = (np.sqrt(std_in * std_in * df * cy0) / np.abs(d)).astype(np.float64)
    lmp = np.broadcast_to(lm, (2, O, N)).reshape(128, N).astype(np.float32)
    eyeb = np.eye(128, dtype=ml_dtypes.bfloat16)
    eyef = np.eye(O, dtype=np.float32)

    in_maps = []
    for c in range(NCORES):
        idxs = c + NCORES * np.arange(FPC)
        dd = d[idxs]
        ss = s_all[idxs]
        phase = np.exp(-1j * om[idxs, None, None] * delay[None, :, :])
        ctf = (w_n[None, :, :] * phase) / dd[:, None, None]   # [FPC,N,N] c128

        def cats(j, scale):
            cm = (ctf[j] * scale).astype(np.complex64)
            c1 = np.concatenate([cm.real, cm.imag], axis=1)
            c2 = np.concatenate([-cm.imag, cm.real], axis=1)
            return c1, c2

        catb = np.zeros((max(len(slot_bf), 1), N, NCAT), ml_dtypes.bfloat16)
        for (j, r, cat), s in slot_bf.items():
            m = degrees[j]
            c1, c2 = cats(j, ss[j] if r == m else 1.0)
            catb[s] = (c1 if cat == 1 else c2).astype(ml_dtypes.bfloat16)
        catf = np.zeros((max(len(slot_fr), 1), N, NCAT), np.float32)
        for (j, sc_, cat), s in slot_fr.items():
            c1, c2 = cats(j, ss[j] if sc_ == "s" else 1.0)
            catf[s] = (c1 if cat == 1 else c2).astype(np.float32)

        slmp = np.zeros((max(len(pairs), 1), 128, N), np.float32)
        for pi, (jA, jB) in enumerate(pairs):
            slmp[pi, 0:O] = ss[jA] * lm
            slmp[pi, O:128] = ss[jB] * lm
        slmb = np.zeros((max(len(solo_bf), 1), O, N), np.float32)
        for si, j in enumerate(solo_bf):
            slmb[si] = ss[j] * lm
        slmf = np.zeros((max(len(solo_f32), 1), O, N), np.float32)
        for si, j in enumerate(solo_f32):
            slmf[si] = ss[j] * lm

        in_maps.append({
            "catb": catb, "catf": catf,
            "lmTb": lmT.astype(ml_dtypes.bfloat16), "lmTf": lmT,
            "lmp": lmp, "slmp": slmp, "slmb": slmb, "slmf": slmf,
            "eyeb": eyeb, "eyef": eyef,
        })

    global LAST
    res = run_bass_kernel_spmd(
        nc, in_maps, core_ids=list(range(NCORES)), trace=PROFILE
    )
    LAST = res
    out = np.zeros((O, O), dtype=np.float64)
    for c in range(NCORES):
        o = res.results[c]["out_e"].astype(np.float64)
        out += o[:, 0:O] + o[:, O:2 * O]
    return out.astype(np.float32)


# revision 8
# speedup vs baseline: 1.2013x; 1.2013x over previous
"""Linearized-Hopf CSD covariance on 8 Trainium2 NeuronCores.

Math: for each frequency f, M(f) = d(f)*I - C(f) with scalar diagonal
d = -a + i*(om - omega0) and C = W .* exp(-i*om*delay), W row-normalized
(row sums == g).  H = M^-1 and the output is
  eeg_cov = cy0*df*std^2 * sum_f Re((lm H)(lm H)^H).
Since ||C/d||_inf = g/|d| < 1, G = lm H is computed by the Horner/
Neumann recurrence  Z <- lm + Z @ Ct  (Ct = C/d, G = Z_m / d), with a
per-frequency degree m chosen from the exact contraction factor
rho = g/|d| for truncation error < tol.  Frequencies are sharded
strided across the 8 cores (core c gets indices c, c+8, ...) so each
core receives the same mix of easy / near-resonance frequencies.

Device mapping (per core):
- Complex products use rhs concatenation: cat1 = [Ctr | Cti] and
  cat2 = [-Cti | Ctr] (N=400), so one stationary pass per component.
- Off-resonance frequencies run in bf16, PAIRED two-per-128-partitions
  (PSUM col-groups 0:64 / 64:128) so matmuls pack into disjoint PE
  column groups and the transposes / combines / copies are shared.
  Their absolute contribution to the summed covariance is 100-1000x
  below the resonant band, so bf16 error is negligible.
- The near-resonance frequencies (deepest recurrences) run solo in
  float32r (full fp32 storage, fast PE mode).
- The output scale s_f = sqrt(std^2*df*cy0)/|d_f| is folded on the
  host into the FINAL round's cat matrices and lm-constant, so the
  channel covariance accumulates in PSUM with no extra scaling ops:
  paired class into a [128,128] block-diagonal accumulator, solo class
  into a [64,64] accumulator; the host sums the halves + 8 cores.
"""

import sys

sys.path.insert(0, "/opt/trn_rl_repo")

import numpy as np
import ml_dtypes

N = 200
O = 64
F = 512
NCORES = 8
FPC = F // NCORES
P_HI = 128
P_LO = N - P_HI
NCAT = 2 * N          # 400
TOL = 1e-3
NSOLO_F32 = 2         # highest-degree positions run solo in f32r

_compiled = {}
PROFILE = False
LAST = None


def _plan(degrees):
    """Group the 64 per-core positions into equal-degree bf16 pairs and
    solo frequencies. Returns (pairs, solo_bf, solo_f32) of position ids,
    highest degree first."""
    order = sorted(range(FPC), key=lambda j: -degrees[j])
    solo_f32 = order[:NSOLO_F32]
    rest = order[NSOLO_F32:]
    pairs, solo_bf = [], []
    by_deg = {}
    for j in rest:
        by_deg.setdefault(degrees[j], []).append(j)
    for deg, js in sorted(by_deg.items(), key=lambda kv: -kv[0]):
        while len(js) >= 2:
            pairs.append((js.pop(0), js.pop(0)))
        if js:
            solo_bf.append(js[0])
    return pairs, solo_bf, solo_f32


def _build_program(degrees):
    import concourse.bacc as bacc
    import concourse.mybir as mybir
    from concourse.tile import TileContext

    f32 = mybir.dt.float32
    f32r = mybir.dt.float32r
    bf16 = mybir.dt.bfloat16

    pairs, solo_bf, solo_f32 = _plan(degrees)

    # DMA slot layout for the bf16 cat tensor: per paired/solo-bf freq,
    # rounds 1..m; round 1 needs only cat1 (Z0 is real); the final round
    # uses the s-scaled cats. Slot key -> index, sized at the end.
    slot_bf = {}   # (pos, round, cat) -> slot index
    for j in [x for p in pairs for x in p] + solo_bf:
        m = degrees[j]
        for r in range(1, m + 1):
            slot_bf[(j, r, 1)] = len(slot_bf)
            if r > 1:
                slot_bf[(j, r, 2)] = len(slot_bf)
    # f32 cats for solo freqs: unscaled cat1/cat2 + scaled cat1/cat2
    slot_fr = {}
    for j in solo_f32:
        m = degrees[j]
        slot_fr[(j, "u", 1)] = len(slot_fr)
        if m > 1:
            slot_fr[(j, "u", 2)] = len(slot_fr)
        slot_fr[(j, "s", 1)] = len(slot_fr)
        if m > 1:
            slot_fr[(j, "s", 2)] = len(slot_fr)

    npair = len(pairs)
    nsbf = len(solo_bf)
    nsfr = len(solo_f32)

    nc = bacc.Bacc(None, target_bir_lowering=False, debug=False)

    catb_d = nc.dram_tensor("catb", [max(len(slot_bf), 1), N, NCAT], bf16,
                            kind="ExternalInput")
    catf_d = nc.dram_tensor("catf", [max(len(slot_fr), 1), N, NCAT], f32r,
                            kind="ExternalInput")
    ctT_d = nc.dram_tensor("ctT", [max(nsfr, 1), 2, N, N], f32r,
                           kind="ExternalInput")
    lmTb_d = nc.dram_tensor("lmTb", [N, O], bf16, kind="ExternalInput")
    lmTf_d = nc.dram_tensor("lmTf", [N, O], f32r, kind="ExternalInput")
    lmp_d = nc.dram_tensor("lmp", [128, N], f32, kind="ExternalInput")
    slmp_d = nc.dram_tensor("slmp", [max(npair, 1), 128, N], f32,
                            kind="ExternalInput")
    slmb_d = nc.dram_tensor("slmb", [max(nsbf, 1), O, N], f32,
                            kind="ExternalInput")
    slmf_d = nc.dram_tensor("slmf", [max(nsfr, 1), O, N], f32,
                            kind="ExternalInput")
    eyeb_d = nc.dram_tensor("eyeb", [128, 128], bf16, kind="ExternalInput")
    eyef_d = nc.dram_tensor("eyef", [128, 128], f32, kind="ExternalInput")
    out_d = nc.dram_tensor("out_e", [O, 2 * O], f32, kind="ExternalOutput")

    with TileContext(nc) as tc:
        with tc.tile_pool(name="consts", bufs=1) as cpool, \
             tc.tile_pool(name="cmatb", bufs=10) as cmatb, \
             tc.tile_pool(name="cmatf", bufs=1) as cmatf, \
             tc.tile_pool(name="zbuf", bufs=4) as zbuf, \
             tc.tile_pool(name="stat", bufs=4) as stat, \
             tc.tile_pool(name="slmbuf", bufs=4) as slmbuf, \
             tc.tile_pool(name="pmm", bufs=4, space="PSUM") as pmm, \
             tc.tile_pool(name="pzt", bufs=3, space="PSUM") as pzt, \
             tc.tile_pool(name="pacc", bufs=1, space="PSUM") as pacc, \
             tc.tile_pool(name="solo_consts", bufs=1) as scpool:

            lmTb_hi = cpool.tile([P_HI, O], bf16)
            nc.sync.dma_start(lmTb_hi[:], lmTb_d[0:P_HI, :])
            lmTb_lo = cpool.tile([P_LO, O], bf16)
            nc.sync.dma_start(lmTb_lo[:], lmTb_d[P_HI:N, :])
            lmTf_hi = cpool.tile([P_HI, O], f32r)
            nc.sync.dma_start(lmTf_hi[:], lmTf_d[0:P_HI, :])
            lmTf_lo = cpool.tile([P_LO, O], f32r)
            nc.sync.dma_start(lmTf_lo[:], lmTf_d[P_HI:N, :])
            lmp_sb = cpool.tile([128, N], f32)
            nc.sync.dma_start(lmp_sb[:], lmp_d[:])
            eyeb_sb = cpool.tile([128, 128], bf16)
            nc.sync.dma_start(eyeb_sb[:], eyeb_d[:])
            eyef_sb128 = cpool.tile([128, 128], f32)
            nc.sync.dma_start(eyef_sb128[:], eyef_d[:])
            eyef_sb = eyef_sb128[0:O, 0:O]
            eyefr_sb128 = cpool.tile([128, 128], f32r)
            nc.gpsimd.dma_start(eyefr_sb128[:], eyef_d[:])
            eyef_sb2 = eyefr_sb128[0:P_HI, 0:P_HI]
            eyef_sb3 = eyefr_sb128[0:P_LO, 0:P_LO]

            pEall = pacc.tile([128, 192], f32)
            pE1 = pEall[:, 0:128]             # paired-class accumulator
            pE2 = pEall[0:O, 128:192]         # solo-f32r accumulator
            e1_state = [True]
            e2_state = [True]
            # count total E-matmuls to set stop on the last one
            ne1_total = (npair + nsbf) * 4
            ne2_total = nsfr * 4
            ne1 = [0]
            ne2 = [0]

            def dma_cat_bf(pos, r, cat):
                s = slot_bf[(pos, r, cat)]
                h = cmatb.tile([P_HI, NCAT], bf16, tag=f"c{cat}h", name=f"cb{cat}h")
                nc.sync.dma_start(h[:], catb_d[s, 0:P_HI, :])
                lo = cmatb.tile([P_LO, NCAT], bf16, tag=f"c{cat}l", name=f"cb{cat}l")
                nc.sync.dma_start(lo[:], catb_d[s, P_HI:N, :])
                return h, lo

            def dma_cat_fr(pos, sc, cat):
                s = slot_fr[(pos, sc, cat)]
                h = cmatf.tile([P_HI, NCAT], f32r, tag=f"f{pos}{sc}{cat}h",
                               name=f"cf{cat}h")
                nc.sync.dma_start(h[:], catf_d[s, 0:P_HI, :])
                lo = cmatf.tile([P_LO, NCAT], f32r, tag=f"f{pos}{sc}{cat}l",
                                name=f"cf{cat}l")
                nc.sync.dma_start(lo[:], catf_d[s, P_HI:N, :])
                return h, lo

            def emit_pair(pi, jA, jB):
                m = degrees[jA]
                ztc = None
                for r in range(1, m + 1):
                    final = r == m
                    p = pmm.tile([128, NCAT], f32, tag="p", name="p")
                    for half, j in ((0, jA), (1, jB)):
                        po = p[64 * half:64 * half + 64, :]
                        c1h, c1l = dma_cat_bf(j, r, 1)
                        if r == 1:
                            nc.tensor.matmul(po, lmTb_hi[:], c1h[:],
                                             start=True, stop=False,
                                             skip_group_check=True)
                            nc.tensor.matmul(po, lmTb_lo[:], c1l[:],
                                             start=False, stop=True,
                                             skip_group_check=True)
                        else:
                            c2h, c2l = dma_cat_bf(j, r, 2)
                            zr_hi = ztc[0][:, 64 * half:64 * half + 64]
                            zr_lo = ztc[1][0:P_LO, 64 * half:64 * half + 64]
                            zi_hi = ztc[2][:, 64 * half:64 * half + 64]
                            zi_lo = ztc[3][0:P_LO, 64 * half:64 * half + 64]
                            nc.tensor.matmul(po, zr_hi, c1h[:], start=True,
                                             stop=False, skip_group_check=True)
                            nc.tensor.matmul(po, zr_lo, c1l[:], start=False,
                                             stop=False, skip_group_check=True)
                            nc.tensor.matmul(po, zi_hi, c2h[:], start=False,
                                             stop=False, skip_group_check=True)
                            nc.tensor.matmul(po, zi_lo, c2l[:], start=False,
                                             stop=True, skip_group_check=True)
                    # combine
                    if final:
                        sl = slmbuf.tile([128, N], f32, tag="slp", name="slp")
                        nc.sync.dma_start(sl[:], slmp_d[pi, :, :])
                    else:
                        sl = lmp_sb
                    zr = zbuf.tile([128, N], bf16, tag="zr", name="zr")
                    nc.vector.tensor_add(zr[:], p[:, 0:N], sl[:])
                    zi = zbuf.tile([128, N], bf16, tag="zi", name="zi")
                    nc.scalar.copy(zi[:], p[:, N:NCAT])
                    # transposes: one PSUM bank, bf16 [128, 512]
                    zt = pzt.tile([128, 512], bf16, tag="zt", name="zt")
                    nc.tensor.transpose(zt[0:P_HI, 0:128], zr[:, 0:P_HI],
                                        eyeb_sb[:])
                    nc.tensor.transpose(zt[0:P_LO, 128:256], zr[:, P_HI:N],
                                        eyeb_sb[:])
                    nc.tensor.transpose(zt[0:P_HI, 256:384], zi[:, 0:P_HI],
                                        eyeb_sb[:])
                    nc.tensor.transpose(zt[0:P_LO, 384:512], zi[:, P_HI:N],
                                        eyeb_sb[:])
                    trh = stat.tile([P_HI, 128], bf16, tag="trh", name="trh")
                    nc.scalar.copy(trh[:], zt[0:P_HI, 0:128])
                    trl = stat.tile([P_LO, 128], bf16, tag="trl", name="trl")
                    nc.vector.tensor_copy(trl[:], zt[0:P_LO, 128:256])
                    tih = stat.tile([P_HI, 128], bf16, tag="tih", name="tih")
                    nc.scalar.copy(tih[:], zt[0:P_HI, 256:384])
                    til = stat.tile([P_LO, 128], bf16, tag="til", name="til")
                    nc.vector.tensor_copy(til[:], zt[0:P_LO, 384:512])
                    ztc = (trh, trl, tih, til)
                # E accumulation: block-diagonal [128,128] += Zt^T Zt
                for t in ztc:
                    kk = t.shape[0] if hasattr(t, "shape") else None
                    ne1[0] += 1
                    nc.tensor.matmul(pE1, t[:], t[:],
                                     start=e1_state[0],
                                     stop=ne1[0] == ne1_total,
                                     skip_group_check=True)
                    e1_state[0] = False

            def emit_solo_bf(si, j):
                # unpaired bf16 frequency: same as pair but half-width
                m = degrees[j]
                ztc = None
                for r in range(1, m + 1):
                    final = r == m
                    p = pmm.tile([128, NCAT], f32, tag="p", name="p")
                    po = p[0:64, :]
                    c1h, c1l = dma_cat_bf(j, r, 1)
                    if r == 1:
                        nc.tensor.matmul(po, lmTb_hi[:], c1h[:], start=True,
                                         stop=False, skip_group_check=True)
                        nc.tensor.matmul(po, lmTb_lo[:], c1l[:], start=False,
                                         stop=True, skip_group_check=True)
                    else:
                        c2h, c2l = dma_cat_bf(j, r, 2)
                        nc.tensor.matmul(po, ztc[0][:, 0:64], c1h[:],
                                         start=True, stop=False,
                                         skip_group_check=True)
                        nc.tensor.matmul(po, ztc[1][0:P_LO, 0:64], c1l[:],
                                         start=False, stop=False,
                                         skip_group_check=True)
                        nc.tensor.matmul(po, ztc[2][:, 0:64], c2h[:],
                                         start=False, stop=False,
                                         skip_group_check=True)
                        nc.tensor.matmul(po, ztc[3][0:P_LO, 0:64], c2l[:],
                                         start=False, stop=True,
                                         skip_group_check=True)
                    if final:
                        sl = slmbuf.tile([O, N], f32, tag="slb", name="slb")
                        nc.sync.dma_start(sl[:], slmb_d[si, :, :])
                        slv = sl[:]
                    else:
                        slv = lmp_sb[0:64, :]
                    zr = zbuf.tile([128, N], bf16, tag="zr", name="zr")
                    nc.vector.tensor_add(zr[0:64, :], p[0:64, 0:N], slv)
                    zi = zbuf.tile([128, N], bf16, tag="zi", name="zi")
                    nc.scalar.copy(zi[0:64, :], p[0:64, N:NCAT])
                    zt = pzt.tile([128, 512], bf16, tag="zt", name="zt")
                    nc.tensor.transpose(zt[0:P_HI, 0:64], zr[0:64, 0:P_HI],
                                        eyeb_sb[0:64, 0:64])
                    nc.tensor.transpose(zt[0:P_LO, 128:192], zr[0:64, P_HI:N],
                                        eyeb_sb[0:64, 0:64])
                    nc.tensor.transpose(zt[0:P_HI, 256:320], zi[0:64, 0:P_HI],
                                        eyeb_sb[0:64, 0:64])
                    nc.tensor.transpose(zt[0:P_LO, 384:448], zi[0:64, P_HI:N],
                                        eyeb_sb[0:64, 0:64])
                    trh = stat.tile([P_HI, 128], bf16, tag="trh", name="trh")
                    nc.scalar.copy(trh[:, 0:64], zt[0:P_HI, 0:64])
                    trl = stat.tile([P_LO, 128], bf16, tag="trl", name="trl")
                    nc.vector.tensor_copy(trl[:, 0:64], zt[0:P_LO, 128:192])
                    tih = stat.tile([P_HI, 128], bf16, tag="tih", name="tih")
                    nc.scalar.copy(tih[:, 0:64], zt[0:P_HI, 256:320])
                    til = stat.tile([P_LO, 128], bf16, tag="til", name="til")
                    nc.vector.tensor_copy(til[:, 0:64], zt[0:P_LO, 384:448])
                    ztc = (trh, trl, tih, til)
                for t in ztc:
                    ne1[0] += 1
                    nc.tensor.matmul(pEall[0:64, 0:64], t[:, 0:64], t[:, 0:64],
                                     start=e1_state[0],
                                     stop=ne1[0] == ne1_total,
                                     skip_group_check=True)
                    e1_state[0] = False

            def solo_round(po, stats, c1, c2, first_round):
                """One f32r Horner round into psum slice po [0:64, NCAT]."""
                (c1h, c1l) = c1
                if first_round:
                    nc.tensor.matmul(po, lmTf_hi[:], c1h[:], start=True,
                                     stop=False, skip_group_check=True)
                    nc.tensor.matmul(po, lmTf_lo[:], c1l[:], start=False,
                                     stop=True, skip_group_check=True)
                else:
                    (c2h, c2l) = c2
                    nc.tensor.matmul(po, stats[0][:], c1h[:], start=True,
                                     stop=False, skip_group_check=True)
                    nc.tensor.matmul(po, stats[1][:], c1l[:], start=False,
                                     stop=False, skip_group_check=True)
                    nc.tensor.matmul(po, stats[2][:], c2h[:], start=False,
                                     stop=False, skip_group_check=True)
                    nc.tensor.matmul(po, stats[3][:], c2l[:], start=False,
                                     stop=True, skip_group_check=True)

            def solo_transpose(zr, zi, to_f32r=True, scoped=stat):
                """Transpose Z [64,200] components into 4 stationaries."""
                zt = pzt.tile([128, 256], f32, tag="zt", name="ztf")
                nc.tensor.transpose(zt[0:P_HI, 0:O], zr[:, 0:P_HI], eyef_sb)
                nc.tensor.transpose(zt[0:P_LO, O:2 * O], zr[:, P_HI:N],
                                    eyef_sb)
                nc.tensor.transpose(zt[0:P_HI, 2 * O:3 * O], zi[:, 0:P_HI],
                                    eyef_sb)
                nc.tensor.transpose(zt[0:P_LO, 3 * O:4 * O], zi[:, P_HI:N],
                                    eyef_sb)
                trh = scoped.tile([P_HI, O], f32r, tag="ftrh", name="ftrh")
                nc.scalar.copy(trh[:], zt[0:P_HI, 0:O])
                trl = scoped.tile([P_LO, O], f32r, tag="ftrl", name="ftrl")
                nc.vector.tensor_copy(trl[:], zt[0:P_LO, O:2 * O])
                tih = scoped.tile([P_HI, O], f32r, tag="ftih", name="ftih")
                nc.scalar.copy(tih[:], zt[0:P_HI, 2 * O:3 * O])
                til = scoped.tile([P_LO, O], f32r, tag="ftil", name="ftil")
                nc.vector.tensor_copy(til[:], zt[0:P_LO, 3 * O:4 * O])
                return (trh, trl, tih, til)

            def build_sq(tag, lhsT_hi_r, lhsT_lo_r, lhsT_hi_i, lhsT_lo_i,
                         rhs1, rhs2):
                """cat tiles of Bsq = B*B from B^T chunk tiles (lhsT_*,
                [kc, 200]) and B's cat tiles (rhs1/rhs2 = (hi,lo))."""
                outs = []
                for mc, (m0, msz) in enumerate(((0, P_HI), (P_HI, P_LO))):
                    p = pmm.tile([128, NCAT], f32, tag="p", name="psq")
                    po = p[0:msz, :]
                    nc.tensor.matmul(po, lhsT_hi_r[:, m0:m0 + msz],
                                     rhs1[0][:], start=True, stop=False,
                                     skip_group_check=True)
                    nc.tensor.matmul(po, lhsT_lo_r[:, m0:m0 + msz],
                                     rhs1[1][:], start=False, stop=False,
                                     skip_group_check=True)
                    nc.tensor.matmul(po, lhsT_hi_i[:, m0:m0 + msz],
                                     rhs2[0][:], start=False, stop=False,
                                     skip_group_check=True)
                    nc.tensor.matmul(po, lhsT_lo_i[:, m0:m0 + msz],
                                     rhs2[1][:], start=False, stop=True,
                                     skip_group_check=True)
                    c1 = scpool.tile([msz, NCAT], f32r, name=f"{tag}c1{mc}")
                    nc.scalar.copy(c1[:], po)
                    c2 = scpool.tile([msz, NCAT], f32r, name=f"{tag}c2{mc}")
                    nc.vector.tensor_scalar_mul(c2[:, 0:N], p[0:msz, N:NCAT],
                                                -1.0)
                    nc.vector.tensor_copy(c2[:, N:NCAT], p[0:msz, 0:N])
                    outs.append((c1, c2, p, msz))
                cat1 = (outs[0][0], outs[1][0])
                cat2 = (outs[0][1], outs[1][1])
                return cat1, cat2

            def transpose_cats(tag, cat1):
                """B^T chunk tiles [kc, 200] x {r,i} from B's cat1 tiles."""
                res = {}
                for comp, coff in (("r", 0), ("i", N)):
                    for kc, (k0, ksz) in enumerate(((0, P_HI), (P_HI, P_LO))):
                        # B^T rows k0:k0+ksz = B cols; gather from both
                        # row-chunks of cat1
                        zt = pzt.tile([128, 256], f32r, tag="zt",
                                      name=f"zt{tag}")
                        nc.tensor.transpose(
                            zt[0:ksz, 0:P_HI],
                            cat1[0][:, coff + k0:coff + k0 + ksz],
                            eyef_sb2)
                        nc.tensor.transpose(
                            zt[0:ksz, P_HI:N],
                            cat1[1][:, coff + k0:coff + k0 + ksz],
                            eyef_sb3)
                        t = scpool.tile([ksz, N], f32r,
                                        name=f"{tag}T{comp}{kc}")
                        nc.scalar.copy(t[:], zt[0:ksz, 0:N])
                        res[(comp, kc)] = t
                return res

            def emit_solo_fr(si, j):
                m = degrees[j]
                cu1 = dma_cat_fr(j, "u", 1) if m > 1 else None
                cu2 = dma_cat_fr(j, "u", 2) if m > 1 else None
                cs1 = dma_cat_fr(j, "s", 1)
                cs2 = dma_cat_fr(j, "s", 2) if m > 1 else None

                use_sq = m >= 8
                if use_sq:
                    ctTr_hi = scpool.tile([P_HI, N], f32r, name=f"cTrh{si}")
                    nc.sync.dma_start(ctTr_hi[:], ctT_d[si, 0, 0:P_HI, :])
                    ctTr_lo = scpool.tile([P_LO, N], f32r, name=f"cTrl{si}")
                    nc.sync.dma_start(ctTr_lo[:], ctT_d[si, 0, P_HI:N, :])
                    ctTi_hi = scpool.tile([P_HI, N], f32r, name=f"cTih{si}")
                    nc.sync.dma_start(ctTi_hi[:], ctT_d[si, 1, 0:P_HI, :])
                    ctTi_lo = scpool.tile([P_LO, N], f32r, name=f"cTil{si}")
                    nc.sync.dma_start(ctTi_lo[:], ctT_d[si, 1, P_HI:N, :])
                    # B2 = Ct^2
                    b2c1, b2c2 = build_sq(f"b2_{si}", ctTr_hi, ctTr_lo,
                                          ctTi_hi, ctTi_lo, cu1, cu2)
                    R = m - 1
                    a4 = (R - 1) // 4
                    rem = R - 1 - 4 * a4
                    b2n = rem // 2
                    c1n = 1 + rem % 2
                    if a4 > 0:
                        b2T = transpose_cats(f"b2_{si}", b2c1)
                        b4c1, b4c2 = build_sq(f"b4_{si}",
                                              b2T[("r", 0)], b2T[("r", 1)],
                                              b2T[("i", 0)], b2T[("i", 1)],
                                              b2c1, b2c2)
                    plan = ([("c1u", None)] * (c1n - 1)
                            + [("b2", None)] * b2n
                            + [("b4", None)] * a4
                            + [("c1s", None)])
                elif m == 1:
                    plan = []
                else:
                    plan = [("c1u", None)] * (m - 2) + [("c1s", None)]

                # round 1 (Z1 = lm + lm*Ct); its output is also A2
                p = pmm.tile([128, NCAT], f32, tag="p", name="p")
                solo_round(p[0:64, :], None, cs1 if m == 1 else cu1, None,
                           True)
                a2r = scpool.tile([O, N], f32, name=f"a2r{si}")
                a2i = scpool.tile([O, N], f32, name=f"a2i{si}")
                if m == 1:
                    sl = slmbuf.tile([O, N], f32, tag="slf", name="slf")
                    nc.sync.dma_start(sl[:], slmf_d[si, :, :])
                    nc.vector.tensor_add(a2r[:], p[0:64, 0:N], sl[:])
                else:
                    nc.vector.tensor_add(a2r[:], p[0:64, 0:N],
                                         lmp_sb[0:64, :])
                nc.scalar.copy(a2i[:], p[0:64, N:NCAT])
                ztc = solo_transpose(a2r, a2i)
                zcur = (a2r, a2i)

                if use_sq:
                    # A4 = A2 + A2 * B2 (reuses round-1 stationaries)
                    p4 = pmm.tile([128, NCAT], f32, tag="p", name="p4")
                    solo_round(p4[0:64, :], ztc, b2c1, b2c2, False)
                    a4r = scpool.tile([O, N], f32, name=f"a4r{si}")
                    nc.vector.tensor_add(a4r[:], p4[0:64, 0:N], a2r[:])
                    a4i = scpool.tile([O, N], f32, name=f"a4i{si}")
                    nc.vector.tensor_add(a4i[:], p4[0:64, N:NCAT], a2i[:])

                for ridx, (kind, _) in enumerate(plan):
                    final = ridx == len(plan) - 1
                    if kind == "c1u":
                        cc1, cc2, addr_, addi_ = cu1, cu2, lmp_sb[0:64, :], None
                    elif kind == "c1s":
                        sl = slmbuf.tile([O, N], f32, tag="slf", name="slf")
                        nc.sync.dma_start(sl[:], slmf_d[si, :, :])
                        cc1, cc2, addr_, addi_ = cs1, cs2, sl[:], None
                    elif kind == "b2":
                        cc1, cc2, addr_, addi_ = b2c1, b2c2, a2r[:], a2i[:]
                    else:
                        cc1, cc2, addr_, addi_ = b4c1, b4c2, a4r[:], a4i[:]
                    p = pmm.tile([128, NCAT], f32, tag="p", name="p")
                    solo_round(p[0:64, :], ztc, cc1, cc2, False)
                    zr = zbuf.tile([O, N], f32, tag="zrf", name="zrf")
                    nc.vector.tensor_add(zr[:], p[0:64, 0:N], addr_)
                    zi = zbuf.tile([O, N], f32, tag="zif", name="zif")
                    if addi_ is None:
                        nc.scalar.copy(zi[:], p[0:64, N:NCAT])
                    else:
                        nc.vector.tensor_add(zi[:], p[0:64, N:NCAT], addi_)
                    ztc = solo_transpose(zr, zi)
                for t in ztc:
                    ne2[0] += 1
                    nc.tensor.matmul(pE2, t[:], t[:],
                                     start=e2_state[0],
                                     stop=ne2[0] == ne2_total,
                                     skip_group_check=True)
                    e2_state[0] = False

            # emit deepest chains first so they overlap everything else
            for si, j in enumerate(solo_f32):
                emit_solo_fr(si, j)
            for si, j in enumerate(solo_bf):
                emit_solo_bf(si, j)
            for pi, (jA, jB) in enumerate(pairs):
                emit_pair(pi, jA, jB)

            e_sb = cpool.tile([O, 2 * O], f32)
            nc.scalar.copy(e_sb[:, O:2 * O], pEall[O:128, O:128])
            nc.vector.tensor_add(e_sb[:, 0:O], pEall[0:O, 0:O],
                                 e_sb[:, O:2 * O])
            nc.scalar.copy(e_sb[:, O:2 * O], pE2)
            nc.sync.dma_start(out_d[:], e_sb[:])

    nc.compile()
    return nc, pairs, solo_bf, solo_f32, slot_bf, slot_fr


def kernel(sc, dist, freqs, lm, wll, a, omega, g, std_in, v_d, cy0):
    from concourse.bass_utils import run_bass_kernel_spmd

    sc = np.asarray(sc, np.float64)
    dist = np.asarray(dist, np.float64)
    freqs32 = np.asarray(freqs, np.float32)
    freqs = freqs32.astype(np.float64)
    lm = np.asarray(lm, np.float32)
    wll = np.asarray(wll, np.float64)
    a = float(a); omega = float(omega); g = float(g)
    std_in = float(std_in); v_d = float(v_d); cy0 = float(cy0)

    w = np.exp(wll) * sc
    w_n = g * w / (w.sum(axis=1, keepdims=True) + 1e-8)
    delay = dist / v_d
    om = 2.0 * np.pi * freqs
    d = -a + 1j * (om - omega)
    df = float(freqs32[1] - freqs32[0])

    rs = w_n.sum(axis=1).max()
    rho = rs / np.abs(d)
    with np.errstate(divide="ignore"):
        mf = np.ceil(np.log(TOL * (1.0 - rho)) / np.log(rho)) - 1.0
    mf = np.clip(np.nan_to_num(mf, nan=1.0), 1, 60).astype(int)

    degrees = tuple(
        int(max(mf[j * NCORES + c] for c in range(NCORES))) for j in range(FPC)
    )
    if degrees not in _compiled:
        _compiled[degrees] = _build_program(degrees)
    nc, pairs, solo_bf, solo_f32, slot_bf, slot_fr = _compiled[degrees]

    lmT = np.ascontiguousarray(lm.T)
    s_all = (np.sqrt(std_in * std_in * df * cy0) / np.abs(d)).astype(np.float64)
    lmp = np.broadcast_to(lm, (2, O, N)).reshape(128, N).astype(np.float32)
    eyeb = np.eye(128, dtype=ml_dtypes.bfloat16)
    eyef = np.eye(128, dtype=np.float32)

    in_maps = []
    for c in range(NCORES):
        idxs = c + NCORES * np.arange(FPC)
        dd = d[idxs]
        ss = s_all[idxs]
        phase = np.exp(-1j * om[idxs, None, None] * delay[None, :, :])
        ctf = (w_n[None, :, :] * phase) / dd[:, None, None]   # [FPC,N,N] c128

        def cats(j, scale):
            cm = (ctf[j] * scale).astype(np.complex64)
            c1 = np.concatenate([cm.real, cm.imag], axis=1)
            c2 = np.concatenate([-cm.imag, cm.real], axis=1)
            return c1, c2

        catb = np.zeros((max(len(slot_bf), 1), N, NCAT), ml_dtypes.bfloat16)
        for (j, r, cat), s in slot_bf.items():
            m = degrees[j]
            c1, c2 = cats(j, ss[j] if r == m else 1.0)
            catb[s] = (c1 if cat == 1 else c2).astype(ml_dtypes.bfloat16)
        ctT = np.zeros((max(len(solo_f32), 1), 2, N, N), np.float32)
        for si, j in enumerate(solo_f32):
            if degrees[j] >= 8:
                ctT[si, 0] = ctf[j].real.T.astype(np.float32)
                ctT[si, 1] = ctf[j].imag.T.astype(np.float32)
        catf = np.zeros((max(len(slot_fr), 1), N, NCAT), np.float32)
        for (j, sc_, cat), s in slot_fr.items():
            c1, c2 = cats(j, ss[j] if sc_ == "s" else 1.0)
            catf[s] = (c1 if cat == 1 else c2).astype(np.float32)

        slmp = np.zeros((max(len(pairs), 1), 128, N), np.float32)
        for pi, (jA, jB) in enumerate(pairs):
            slmp[pi, 0:O] = ss[jA] * lm
            slmp[pi, O:128] = ss[jB] * lm
        slmb = np.zeros((max(len(solo_bf), 1), O, N), np.float32)
        for si, j in enumerate(solo_bf):
            slmb[si] = ss[j] * lm
        slmf = np.zeros((max(len(solo_f32), 1), O, N), np.float32)
        for si, j in enumerate(solo_f32):
            slmf[si] = ss[j] * lm

        in_maps.append({
            "catb": catb, "catf": catf, "ctT": ctT,
            "lmTb": lmT.astype(ml_dtypes.bfloat16), "lmTf": lmT,
            "lmp": lmp, "slmp": slmp, "slmb": slmb, "slmf": slmf,
            "eyeb": eyeb, "eyef": eyef,
        })

    global LAST
    res = run_bass_kernel_spmd(
        nc, in_maps, core_ids=list(range(NCORES)), trace=PROFILE
    )
    LAST = res
    out = np.zeros((O, O), dtype=np.float64)
    for c in range(NCORES):
        o = res.results[c]["out_e"].astype(np.float64)
        out += o[:, 0:O] + o[:, O:2 * O]
    return out.astype(np.float32)


# revision 9
# speedup vs baseline: 1.2151x; 1.0114x over previous
"""Linearized-Hopf CSD covariance on 8 Trainium2 NeuronCores.

Math: for each frequency f, M(f) = d(f)*I - C(f) with scalar diagonal
d = -a + i*(om - omega0) and C = W .* exp(-i*om*delay), W row-normalized
(row sums == g).  H = M^-1 and the output is
  eeg_cov = cy0*df*std^2 * sum_f Re((lm H)(lm H)^H).
Since ||C/d||_inf = g/|d| < 1, G = lm H is computed by the Horner/
Neumann recurrence  Z <- lm + Z @ Ct  (Ct = C/d, G = Z_m / d), with a
per-frequency degree m chosen from the exact contraction factor
rho = g/|d| for truncation error < tol.  Frequencies are sharded
strided across the 8 cores (core c gets indices c, c+8, ...) so each
core receives the same mix of easy / near-resonance frequencies.

Device mapping (per core):
- Complex products use rhs concatenation: cat1 = [Ctr | Cti] and
  cat2 = [-Cti | Ctr] (N=400), so one stationary pass per component.
- Off-resonance frequencies run in bf16, PAIRED two-per-128-partitions
  (PSUM col-groups 0:64 / 64:128) so matmuls pack into disjoint PE
  column groups and the transposes / combines / copies are shared.
  Their absolute contribution to the summed covariance is 100-1000x
  below the resonant band, so bf16 error is negligible.
- The near-resonance frequencies (deepest recurrences) run solo in
  float32r (full fp32 storage, fast PE mode).
- The output scale s_f = sqrt(std^2*df*cy0)/|d_f| is folded on the
  host into the FINAL round's cat matrices and lm-constant, so the
  channel covariance accumulates in PSUM with no extra scaling ops:
  paired class into a [128,128] block-diagonal accumulator, solo class
  into a [64,64] accumulator; the host sums the halves + 8 cores.
"""

import sys

sys.path.insert(0, "/opt/trn_rl_repo")

import numpy as np
import ml_dtypes

N = 200
O = 64
F = 512
NCORES = 8
FPC = F // NCORES
P_HI = 128
P_LO = N - P_HI
NCAT = 2 * N          # 400
TOL = 1e-3
NSOLO_F32 = 2         # highest-degree positions run solo in f32r

_compiled = {}
PROFILE = False
LAST = None


def _plan(degrees):
    """Group the 64 per-core positions into equal-degree bf16 pairs and
    solo frequencies. Returns (pairs, solo_bf, solo_f32) of position ids,
    highest degree first."""
    order = sorted(range(FPC), key=lambda j: -degrees[j])
    solo_f32 = order[:NSOLO_F32]
    rest = order[NSOLO_F32:]
    pairs, solo_bf = [], []
    by_deg = {}
    for j in rest:
        by_deg.setdefault(degrees[j], []).append(j)
    for deg, js in sorted(by_deg.items(), key=lambda kv: -kv[0]):
        while len(js) >= 2:
            pairs.append((js.pop(0), js.pop(0)))
        if js:
            solo_bf.append(js[0])
    return pairs, solo_bf, solo_f32


def _build_program(degrees):
    import concourse.bacc as bacc
    import concourse.mybir as mybir
    from concourse.tile import TileContext

    f32 = mybir.dt.float32
    f32r = mybir.dt.float32r
    bf16 = mybir.dt.bfloat16

    pairs, solo_bf, solo_f32 = _plan(degrees)

    # DMA slot layout for the bf16 cat tensor: per paired/solo-bf freq,
    # rounds 1..m; round 1 needs only cat1 (Z0 is real); the final round
    # uses the s-scaled cats. Slot key -> index, sized at the end.
    slot_bf = {}   # (pos, round, cat) -> slot index
    for j in [x for p in pairs for x in p] + solo_bf:
        m = degrees[j]
        for r in range(1, m + 1):
            slot_bf[(j, r, 1)] = len(slot_bf)
            if r > 1:
                slot_bf[(j, r, 2)] = len(slot_bf)
    # f32 cats for solo freqs: unscaled cat1/cat2 + scaled cat1/cat2
    slot_fr = {}
    for j in solo_f32:
        m = degrees[j]
        slot_fr[(j, "u", 1)] = len(slot_fr)
        if m > 1:
            slot_fr[(j, "u", 2)] = len(slot_fr)
        slot_fr[(j, "s", 1)] = len(slot_fr)
        if m > 1:
            slot_fr[(j, "s", 2)] = len(slot_fr)

    npair = len(pairs)
    nsbf = len(solo_bf)
    nsfr = len(solo_f32)

    nc = bacc.Bacc(None, target_bir_lowering=False, debug=False)

    catb_d = nc.dram_tensor("catb", [max(len(slot_bf), 1), N, NCAT], bf16,
                            kind="ExternalInput")
    catf_d = nc.dram_tensor("catf", [max(len(slot_fr), 1), N, NCAT], f32r,
                            kind="ExternalInput")
    ctT_d = nc.dram_tensor("ctT", [max(nsfr, 1), 2, N, N], f32r,
                           kind="ExternalInput")
    lmTb_d = nc.dram_tensor("lmTb", [N, O], bf16, kind="ExternalInput")
    lmTf_d = nc.dram_tensor("lmTf", [N, O], f32r, kind="ExternalInput")
    lmp_d = nc.dram_tensor("lmp", [128, N], f32, kind="ExternalInput")
    slmp_d = nc.dram_tensor("slmp", [max(npair, 1), 128, N], f32,
                            kind="ExternalInput")
    slmb_d = nc.dram_tensor("slmb", [max(nsbf, 1), O, N], f32,
                            kind="ExternalInput")
    slmf_d = nc.dram_tensor("slmf", [max(nsfr, 1), O, N], f32,
                            kind="ExternalInput")
    eyeb_d = nc.dram_tensor("eyeb", [128, 128], bf16, kind="ExternalInput")
    eyef_d = nc.dram_tensor("eyef", [128, 128], f32, kind="ExternalInput")
    out_d = nc.dram_tensor("out_e", [O, 2 * O], f32, kind="ExternalOutput")

    with TileContext(nc) as tc:
        with tc.tile_pool(name="consts", bufs=1) as cpool, \
             tc.tile_pool(name="cmatb", bufs=12) as cmatb, \
             tc.tile_pool(name="cmatf", bufs=1) as cmatf, \
             tc.tile_pool(name="zbuf", bufs=6) as zbuf, \
             tc.tile_pool(name="stat", bufs=6) as stat, \
             tc.tile_pool(name="slmbuf", bufs=4) as slmbuf, \
             tc.tile_pool(name="pmm", bufs=4, space="PSUM") as pmm, \
             tc.tile_pool(name="pzt", bufs=3, space="PSUM") as pzt, \
             tc.tile_pool(name="pacc", bufs=1, space="PSUM") as pacc, \
             tc.tile_pool(name="solo_consts", bufs=1) as scpool:

            lmTb_hi = cpool.tile([P_HI, O], bf16)
            nc.sync.dma_start(lmTb_hi[:], lmTb_d[0:P_HI, :])
            lmTb_lo = cpool.tile([P_LO, O], bf16)
            nc.sync.dma_start(lmTb_lo[:], lmTb_d[P_HI:N, :])
            lmTf_hi = cpool.tile([P_HI, O], f32r)
            nc.sync.dma_start(lmTf_hi[:], lmTf_d[0:P_HI, :])
            lmTf_lo = cpool.tile([P_LO, O], f32r)
            nc.sync.dma_start(lmTf_lo[:], lmTf_d[P_HI:N, :])
            lmp_sb = cpool.tile([128, N], f32)
            nc.sync.dma_start(lmp_sb[:], lmp_d[:])
            eyeb_sb = cpool.tile([128, 128], bf16)
            nc.sync.dma_start(eyeb_sb[:], eyeb_d[:])
            eyef_sb128 = cpool.tile([128, 128], f32)
            nc.sync.dma_start(eyef_sb128[:], eyef_d[:])
            eyef_sb = eyef_sb128[0:O, 0:O]
            eyefr_sb128 = cpool.tile([128, 128], f32r)
            nc.gpsimd.dma_start(eyefr_sb128[:], eyef_d[:])
            eyef_sb2 = eyefr_sb128[0:P_HI, 0:P_HI]
            eyef_sb3 = eyefr_sb128[0:P_LO, 0:P_LO]

            pEall = pacc.tile([128, 192], f32)
            pE1 = pEall[:, 0:128]             # paired-class accumulator
            pE2 = pEall[0:O, 128:192]         # solo-f32r accumulator
            e1_state = [True]
            e2_state = [True]
            # count total E-matmuls to set stop on the last one
            ne1_total = (npair + nsbf) * 4
            ne2_total = nsfr * 4
            ne1 = [0]
            ne2 = [0]

            def dma_cat_bf(pos, r, cat):
                s = slot_bf[(pos, r, cat)]
                h = cmatb.tile([P_HI, NCAT], bf16, tag=f"c{cat}h", name=f"cb{cat}h")
                nc.sync.dma_start(h[:], catb_d[s, 0:P_HI, :])
                lo = cmatb.tile([P_LO, NCAT], bf16, tag=f"c{cat}l", name=f"cb{cat}l")
                nc.sync.dma_start(lo[:], catb_d[s, P_HI:N, :])
                return h, lo

            def dma_cat_fr(pos, sc, cat):
                s = slot_fr[(pos, sc, cat)]
                h = cmatf.tile([P_HI, NCAT], f32r, tag=f"f{pos}{sc}{cat}h",
                               name=f"cf{cat}h")
                nc.sync.dma_start(h[:], catf_d[s, 0:P_HI, :])
                lo = cmatf.tile([P_LO, NCAT], f32r, tag=f"f{pos}{sc}{cat}l",
                                name=f"cf{cat}l")
                nc.sync.dma_start(lo[:], catf_d[s, P_HI:N, :])
                return h, lo

            def emit_pair(pi, jA, jB):
                m = degrees[jA]
                ztc = None
                for r in range(1, m + 1):
                    final = r == m
                    p = pmm.tile([128, NCAT], f32, tag="p", name="p")
                    for half, j in ((0, jA), (1, jB)):
                        po = p[64 * half:64 * half + 64, :]
                        c1h, c1l = dma_cat_bf(j, r, 1)
                        if r == 1:
                            nc.tensor.matmul(po, lmTb_hi[:], c1h[:],
                                             start=True, stop=False,
                                             skip_group_check=True)
                            nc.tensor.matmul(po, lmTb_lo[:], c1l[:],
                                             start=False, stop=True,
                                             skip_group_check=True)
                        else:
                            c2h, c2l = dma_cat_bf(j, r, 2)
                            zr_hi = ztc[0][:, 64 * half:64 * half + 64]
                            zr_lo = ztc[1][0:P_LO, 64 * half:64 * half + 64]
                            zi_hi = ztc[2][:, 64 * half:64 * half + 64]
                            zi_lo = ztc[3][0:P_LO, 64 * half:64 * half + 64]
                            nc.tensor.matmul(po, zr_hi, c1h[:], start=True,
                                             stop=False, skip_group_check=True)
                            nc.tensor.matmul(po, zr_lo, c1l[:], start=False,
                                             stop=False, skip_group_check=True)
                            nc.tensor.matmul(po, zi_hi, c2h[:], start=False,
                                             stop=False, skip_group_check=True)
                            nc.tensor.matmul(po, zi_lo, c2l[:], start=False,
                                             stop=True, skip_group_check=True)
                    # combine
                    if final:
                        sl = slmbuf.tile([128, N], f32, tag="slp", name="slp")
                        nc.sync.dma_start(sl[:], slmp_d[pi, :, :])
                    else:
                        sl = lmp_sb
                    zr = zbuf.tile([128, N], bf16, tag="zr", name="zr")
                    nc.vector.tensor_add(zr[:], p[:, 0:N], sl[:])
                    zi = zbuf.tile([128, N], bf16, tag="zi", name="zi")
                    nc.scalar.copy(zi[:], p[:, N:NCAT])
                    # transposes: one PSUM bank, bf16 [128, 512]
                    zt = pzt.tile([128, 512], bf16, tag="zt", name="zt")
                    nc.tensor.transpose(zt[0:P_HI, 0:128], zr[:, 0:P_HI],
                                        eyeb_sb[:])
                    nc.tensor.transpose(zt[0:P_LO, 128:256], zr[:, P_HI:N],
                                        eyeb_sb[:])
                    nc.tensor.transpose(zt[0:P_HI, 256:384], zi[:, 0:P_HI],
                                        eyeb_sb[:])
                    nc.tensor.transpose(zt[0:P_LO, 384:512], zi[:, P_HI:N],
                                        eyeb_sb[:])
                    trh = stat.tile([P_HI, 128], bf16, tag="trh", name="trh")
                    nc.scalar.copy(trh[:], zt[0:P_HI, 0:128])
                    trl = stat.tile([P_LO, 128], bf16, tag="trl", name="trl")
                    nc.vector.tensor_copy(trl[:], zt[0:P_LO, 128:256])
                    tih = stat.tile([P_HI, 128], bf16, tag="tih", name="tih")
                    nc.vector.tensor_copy(tih[:], zt[0:P_HI, 256:384])
                    til = stat.tile([P_LO, 128], bf16, tag="til", name="til")
                    nc.vector.tensor_copy(til[:], zt[0:P_LO, 384:512])
                    ztc = (trh, trl, tih, til)
                # E accumulation: block-diagonal [128,128] += Zt^T Zt
                for t in ztc:
                    kk = t.shape[0] if hasattr(t, "shape") else None
                    ne1[0] += 1
                    nc.tensor.matmul(pE1, t[:], t[:],
                                     start=e1_state[0],
                                     stop=ne1[0] == ne1_total,
                                     skip_group_check=True)
                    e1_state[0] = False

            def emit_solo_bf(si, j):
                # unpaired bf16 frequency: same as pair but half-width
                m = degrees[j]
                ztc = None
                for r in range(1, m + 1):
                    final = r == m
                    p = pmm.tile([128, NCAT], f32, tag="p", name="p")
                    po = p[0:64, :]
                    c1h, c1l = dma_cat_bf(j, r, 1)
                    if r == 1:
                        nc.tensor.matmul(po, lmTb_hi[:], c1h[:], start=True,
                                         stop=False, skip_group_check=True)
                        nc.tensor.matmul(po, lmTb_lo[:], c1l[:], start=False,
                                         stop=True, skip_group_check=True)
                    else:
                        c2h, c2l = dma_cat_bf(j, r, 2)
                        nc.tensor.matmul(po, ztc[0][:, 0:64], c1h[:],
                                         start=True, stop=False,
                                         skip_group_check=True)
                        nc.tensor.matmul(po, ztc[1][0:P_LO, 0:64], c1l[:],
                                         start=False, stop=False,
                                         skip_group_check=True)
                        nc.tensor.matmul(po, ztc[2][:, 0:64], c2h[:],
                                         start=False, stop=False,
                                         skip_group_check=True)
                        nc.tensor.matmul(po, ztc[3][0:P_LO, 0:64], c2l[:],
                                         start=False, stop=True,
                                         skip_group_check=True)
                    if final:
                        sl = slmbuf.tile([O, N], f32, tag="slb", name="slb")
                        nc.sync.dma_start(sl[:], slmb_d[si, :, :])
                        slv = sl[:]
                    else:
                        slv = lmp_sb[0:64, :]
                    zr = zbuf.tile([128, N], bf16, tag="zr", name="zr")
                    nc.vector.tensor_add(zr[0:64, :], p[0:64, 0:N], slv)
                    zi = zbuf.tile([128, N], bf16, tag="zi", name="zi")
                    nc.scalar.copy(zi[0:64, :], p[0:64, N:NCAT])
                    zt = pzt.tile([128, 512], bf16, tag="zt", name="zt")
                    nc.tensor.transpose(zt[0:P_HI, 0:64], zr[0:64, 0:P_HI],
                                        eyeb_sb[0:64, 0:64])
                    nc.tensor.transpose(zt[0:P_LO, 128:192], zr[0:64, P_HI:N],
                                        eyeb_sb[0:64, 0:64])
                    nc.tensor.transpose(zt[0:P_HI, 256:320], zi[0:64, 0:P_HI],
                                        eyeb_sb[0:64, 0:64])
                    nc.tensor.transpose(zt[0:P_LO, 384:448], zi[0:64, P_HI:N],
                                        eyeb_sb[0:64, 0:64])
                    trh = stat.tile([P_HI, 128], bf16, tag="trh", name="trh")
                    nc.scalar.copy(trh[:, 0:64], zt[0:P_HI, 0:64])
                    trl = stat.tile([P_LO, 128], bf16, tag="trl", name="trl")
                    nc.vector.tensor_copy(trl[:, 0:64], zt[0:P_LO, 128:192])
                    tih = stat.tile([P_HI, 128], bf16, tag="tih", name="tih")
                    nc.scalar.copy(tih[:, 0:64], zt[0:P_HI, 256:320])
                    til = stat.tile([P_LO, 128], bf16, tag="til", name="til")
                    nc.vector.tensor_copy(til[:, 0:64], zt[0:P_LO, 384:448])
                    ztc = (trh, trl, tih, til)
                for t in ztc:
                    ne1[0] += 1
                    nc.tensor.matmul(pEall[0:64, 0:64], t[:, 0:64], t[:, 0:64],
                                     start=e1_state[0],
                                     stop=ne1[0] == ne1_total,
                                     skip_group_check=True)
                    e1_state[0] = False

            def solo_round(po, stats, c1, c2, first_round):
                """One f32r Horner round into psum slice po [0:64, NCAT]."""
                (c1h, c1l) = c1
                if first_round:
                    nc.tensor.matmul(po, lmTf_hi[:], c1h[:], start=True,
                                     stop=False, skip_group_check=True)
                    nc.tensor.matmul(po, lmTf_lo[:], c1l[:], start=False,
                                     stop=True, skip_group_check=True)
                else:
                    (c2h, c2l) = c2
                    nc.tensor.matmul(po, stats[0][:], c1h[:], start=True,
                                     stop=False, skip_group_check=True)
                    nc.tensor.matmul(po, stats[1][:], c1l[:], start=False,
                                     stop=False, skip_group_check=True)
                    nc.tensor.matmul(po, stats[2][:], c2h[:], start=False,
                                     stop=False, skip_group_check=True)
                    nc.tensor.matmul(po, stats[3][:], c2l[:], start=False,
                                     stop=True, skip_group_check=True)

            def solo_transpose(zr, zi, to_f32r=True, scoped=stat):
                """Transpose Z [64,200] components into 4 stationaries."""
                zt = pzt.tile([128, 256], f32, tag="zt", name="ztf")
                nc.tensor.transpose(zt[0:P_HI, 0:O], zr[:, 0:P_HI], eyef_sb)
                nc.tensor.transpose(zt[0:P_LO, O:2 * O], zr[:, P_HI:N],
                                    eyef_sb)
                nc.tensor.transpose(zt[0:P_HI, 2 * O:3 * O], zi[:, 0:P_HI],
                                    eyef_sb)
                nc.tensor.transpose(zt[0:P_LO, 3 * O:4 * O], zi[:, P_HI:N],
                                    eyef_sb)
                trh = scoped.tile([P_HI, O], f32r, tag="ftrh", name="ftrh")
                nc.scalar.copy(trh[:], zt[0:P_HI, 0:O])
                trl = scoped.tile([P_LO, O], f32r, tag="ftrl", name="ftrl")
                nc.vector.tensor_copy(trl[:], zt[0:P_LO, O:2 * O])
                tih = scoped.tile([P_HI, O], f32r, tag="ftih", name="ftih")
                nc.scalar.copy(tih[:], zt[0:P_HI, 2 * O:3 * O])
                til = scoped.tile([P_LO, O], f32r, tag="ftil", name="ftil")
                nc.vector.tensor_copy(til[:], zt[0:P_LO, 3 * O:4 * O])
                return (trh, trl, tih, til)

            def build_sq(tag, lhsT_hi_r, lhsT_lo_r, lhsT_hi_i, lhsT_lo_i,
                         rhs1, rhs2):
                """cat tiles of Bsq = B*B from B^T chunk tiles (lhsT_*,
                [kc, 200]) and B's cat tiles (rhs1/rhs2 = (hi,lo))."""
                outs = []
                for mc, (m0, msz) in enumerate(((0, P_HI), (P_HI, P_LO))):
                    p = pmm.tile([128, NCAT], f32, tag="p", name="psq")
                    po = p[0:msz, :]
                    nc.tensor.matmul(po, lhsT_hi_r[:, m0:m0 + msz],
                                     rhs1[0][:], start=True, stop=False,
                                     skip_group_check=True)
                    nc.tensor.matmul(po, lhsT_lo_r[:, m0:m0 + msz],
                                     rhs1[1][:], start=False, stop=False,
                                     skip_group_check=True)
                    nc.tensor.matmul(po, lhsT_hi_i[:, m0:m0 + msz],
                                     rhs2[0][:], start=False, stop=False,
                                     skip_group_check=True)
                    nc.tensor.matmul(po, lhsT_lo_i[:, m0:m0 + msz],
                                     rhs2[1][:], start=False, stop=True,
                                     skip_group_check=True)
                    c1 = scpool.tile([msz, NCAT], f32r, name=f"{tag}c1{mc}")
                    nc.scalar.copy(c1[:], po)
                    c2 = scpool.tile([msz, NCAT], f32r, name=f"{tag}c2{mc}")
                    nc.vector.tensor_scalar_mul(c2[:, 0:N], p[0:msz, N:NCAT],
                                                -1.0)
                    nc.vector.tensor_copy(c2[:, N:NCAT], p[0:msz, 0:N])
                    outs.append((c1, c2, p, msz))
                cat1 = (outs[0][0], outs[1][0])
                cat2 = (outs[0][1], outs[1][1])
                return cat1, cat2

            def transpose_cats(tag, cat1):
                """B^T chunk tiles [kc, 200] x {r,i} from B's cat1 tiles."""
                res = {}
                for comp, coff in (("r", 0), ("i", N)):
                    for kc, (k0, ksz) in enumerate(((0, P_HI), (P_HI, P_LO))):
                        # B^T rows k0:k0+ksz = B cols; gather from both
                        # row-chunks of cat1
                        zt = pzt.tile([128, 256], f32r, tag="zt",
                                      name=f"zt{tag}")
                        nc.tensor.transpose(
                            zt[0:ksz, 0:P_HI],
                            cat1[0][:, coff + k0:coff + k0 + ksz],
                            eyef_sb2)
                        nc.tensor.transpose(
                            zt[0:ksz, P_HI:N],
                            cat1[1][:, coff + k0:coff + k0 + ksz],
                            eyef_sb3)
                        t = scpool.tile([ksz, N], f32r,
                                        name=f"{tag}T{comp}{kc}")
                        nc.scalar.copy(t[:], zt[0:ksz, 0:N])
                        res[(comp, kc)] = t
                return res

            def emit_solo_fr(si, j):
                m = degrees[j]
                cu1 = dma_cat_fr(j, "u", 1) if m > 1 else None
                cu2 = dma_cat_fr(j, "u", 2) if m > 1 else None
                cs1 = dma_cat_fr(j, "s", 1)
                cs2 = dma_cat_fr(j, "s", 2) if m > 1 else None

                use_sq = m >= 8
                if use_sq:
                    ctTr_hi = scpool.tile([P_HI, N], f32r, name=f"cTrh{si}")
                    nc.sync.dma_start(ctTr_hi[:], ctT_d[si, 0, 0:P_HI, :])
                    ctTr_lo = scpool.tile([P_LO, N], f32r, name=f"cTrl{si}")
                    nc.sync.dma_start(ctTr_lo[:], ctT_d[si, 0, P_HI:N, :])
                    ctTi_hi = scpool.tile([P_HI, N], f32r, name=f"cTih{si}")
                    nc.sync.dma_start(ctTi_hi[:], ctT_d[si, 1, 0:P_HI, :])
                    ctTi_lo = scpool.tile([P_LO, N], f32r, name=f"cTil{si}")
                    nc.sync.dma_start(ctTi_lo[:], ctT_d[si, 1, P_HI:N, :])
                    # B2 = Ct^2
                    b2c1, b2c2 = build_sq(f"b2_{si}", ctTr_hi, ctTr_lo,
                                          ctTi_hi, ctTi_lo, cu1, cu2)
                    R = m - 1
                    a4 = (R - 1) // 4
                    rem = R - 1 - 4 * a4
                    b2n = rem // 2
                    c1n = 1 + rem % 2
                    if a4 > 0:
                        b2T = transpose_cats(f"b2_{si}", b2c1)
                        b4c1, b4c2 = build_sq(f"b4_{si}",
                                              b2T[("r", 0)], b2T[("r", 1)],
                                              b2T[("i", 0)], b2T[("i", 1)],
                                              b2c1, b2c2)
                    plan = ([("c1u", None)] * (c1n - 1)
                            + [("b2", None)] * b2n
                            + [("b4", None)] * a4
                            + [("c1s", None)])
                elif m == 1:
                    plan = []
                else:
                    plan = [("c1u", None)] * (m - 2) + [("c1s", None)]

                # round 1 (Z1 = lm + lm*Ct); its output is also A2
                p = pmm.tile([128, NCAT], f32, tag="p", name="p")
                solo_round(p[0:64, :], None, cs1 if m == 1 else cu1, None,
                           True)
                a2r = scpool.tile([O, N], f32, name=f"a2r{si}")
                a2i = scpool.tile([O, N], f32, name=f"a2i{si}")
                if m == 1:
                    sl = slmbuf.tile([O, N], f32, tag="slf", name="slf")
                    nc.sync.dma_start(sl[:], slmf_d[si, :, :])
                    nc.vector.tensor_add(a2r[:], p[0:64, 0:N], sl[:])
                else:
                    nc.vector.tensor_add(a2r[:], p[0:64, 0:N],
                                         lmp_sb[0:64, :])
                nc.scalar.copy(a2i[:], p[0:64, N:NCAT])
                ztc = solo_transpose(a2r, a2i)
                zcur = (a2r, a2i)

                if use_sq:
                    # A4 = A2 + A2 * B2 (reuses round-1 stationaries)
                    p4 = pmm.tile([128, NCAT], f32, tag="p", name="p4")
                    solo_round(p4[0:64, :], ztc, b2c1, b2c2, False)
                    a4r = scpool.tile([O, N], f32, name=f"a4r{si}")
                    nc.vector.tensor_add(a4r[:], p4[0:64, 0:N], a2r[:])
                    a4i = scpool.tile([O, N], f32, name=f"a4i{si}")
                    nc.vector.tensor_add(a4i[:], p4[0:64, N:NCAT], a2i[:])

                for ridx, (kind, _) in enumerate(plan):
                    final = ridx == len(plan) - 1
                    if kind == "c1u":
                        cc1, cc2, addr_, addi_ = cu1, cu2, lmp_sb[0:64, :], None
                    elif kind == "c1s":
                        sl = slmbuf.tile([O, N], f32, tag="slf", name="slf")
                        nc.sync.dma_start(sl[:], slmf_d[si, :, :])
                        cc1, cc2, addr_, addi_ = cs1, cs2, sl[:], None
                    elif kind == "b2":
                        cc1, cc2, addr_, addi_ = b2c1, b2c2, a2r[:], a2i[:]
                    else:
                        cc1, cc2, addr_, addi_ = b4c1, b4c2, a4r[:], a4i[:]
                    p = pmm.tile([128, NCAT], f32, tag="p", name="p")
                    solo_round(p[0:64, :], ztc, cc1, cc2, False)
                    zr = zbuf.tile([O, N], f32, tag="zrf", name="zrf")
                    nc.vector.tensor_add(zr[:], p[0:64, 0:N], addr_)
                    zi = zbuf.tile([O, N], f32, tag="zif", name="zif")
                    if addi_ is None:
                        nc.scalar.copy(zi[:], p[0:64, N:NCAT])
                    else:
                        nc.vector.tensor_add(zi[:], p[0:64, N:NCAT], addi_)
                    ztc = solo_transpose(zr, zi)
                for t in ztc:
                    ne2[0] += 1
                    nc.tensor.matmul(pE2, t[:], t[:],
                                     start=e2_state[0],
                                     stop=ne2[0] == ne2_total,
                                     skip_group_check=True)
                    e2_state[0] = False

            # a few cheap bf16 pairs first (their small DMAs land fast and
            # warm the PE while the big solo-chain f32 cats stream in), then
            # the deep solo chains, then everything else
            NWARM = 3
            for pi in range(min(NWARM, len(pairs))):
                emit_pair(pi, *pairs[pi])
            for si, j in enumerate(solo_f32):
                emit_solo_fr(si, j)
            for si, j in enumerate(solo_bf):
                emit_solo_bf(si, j)
            for pi in range(min(NWARM, len(pairs)), len(pairs)):
                emit_pair(pi, *pairs[pi])

            e_sb = cpool.tile([O, 2 * O], f32)
            nc.scalar.copy(e_sb[:, O:2 * O], pEall[O:128, O:128])
            nc.vector.tensor_add(e_sb[:, 0:O], pEall[0:O, 0:O],
                                 e_sb[:, O:2 * O])
            nc.scalar.copy(e_sb[:, O:2 * O], pE2)
            nc.sync.dma_start(out_d[:], e_sb[:])

    nc.compile()
    return nc, pairs, solo_bf, solo_f32, slot_bf, slot_fr


def kernel(sc, dist, freqs, lm, wll, a, omega, g, std_in, v_d, cy0):
    from concourse.bass_utils import run_bass_kernel_spmd

    sc = np.asarray(sc, np.float64)
    dist = np.asarray(dist, np.float64)
    freqs32 = np.asarray(freqs, np.float32)
    freqs = freqs32.astype(np.float64)
    lm = np.asarray(lm, np.float32)
    wll = np.asarray(wll, np.float64)
    a = float(a); omega = float(omega); g = float(g)
    std_in = float(std_in); v_d = float(v_d); cy0 = float(cy0)

    w = np.exp(wll) * sc
    w_n = g * w / (w.sum(axis=1, keepdims=True) + 1e-8)
    delay = dist / v_d
    om = 2.0 * np.pi * freqs
    d = -a + 1j * (om - omega)
    df = float(freqs32[1] - freqs32[0])

    rs = w_n.sum(axis=1).max()
    rho = rs / np.abs(d)
    with np.errstate(divide="ignore"):
        mf = np.ceil(np.log(TOL * (1.0 - rho)) / np.log(rho)) - 1.0
    mf = np.clip(np.nan_to_num(mf, nan=1.0), 1, 60).astype(int)

    degrees = tuple(
        int(max(mf[j * NCORES + c] for c in range(NCORES))) for j in range(FPC)
    )
    if degrees not in _compiled:
        _compiled[degrees] = _build_program(degrees)
    nc, pairs, solo_bf, solo_f32, slot_bf, slot_fr = _compiled[degrees]

    lmT = np.ascontiguousarray(lm.T)
    s_all = (np.sqrt(std_in * std_in * df * cy0) / np.abs(d)).astype(np.float64)
    lmp = np.broadcast_to(lm, (2, O, N)).reshape(128, N).astype(np.float32)
    eyeb = np.eye(128, dtype=ml_dtypes.bfloat16)
    eyef = np.eye(128, dtype=np.float32)

    in_maps = []
    for c in range(NCORES):
        idxs = c + NCORES * np.arange(FPC)
        dd = d[idxs]
        ss = s_all[idxs]
        phase = np.exp(-1j * om[idxs, None, None] * delay[None, :, :])
        ctf = (w_n[None, :, :] * phase) / dd[:, None, None]   # [FPC,N,N] c128

        def cats(j, scale):
            cm = (ctf[j] * scale).astype(np.complex64)
            c1 = np.concatenate([cm.real, cm.imag], axis=1)
            c2 = np.concatenate([-cm.imag, cm.real], axis=1)
            return c1, c2

        catb = np.zeros((max(len(slot_bf), 1), N, NCAT), ml_dtypes.bfloat16)
        for (j, r, cat), s in slot_bf.items():
            m = degrees[j]
            c1, c2 = cats(j, ss[j] if r == m else 1.0)
            catb[s] = (c1 if cat == 1 else c2).astype(ml_dtypes.bfloat16)
        ctT = np.zeros((max(len(solo_f32), 1), 2, N, N), np.float32)
        for si, j in enumerate(solo_f32):
            if degrees[j] >= 8:
                ctT[si, 0] = ctf[j].real.T.astype(np.float32)
                ctT[si, 1] = ctf[j].imag.T.astype(np.float32)
        catf = np.zeros((max(len(slot_fr), 1), N, NCAT), np.float32)
        for (j, sc_, cat), s in slot_fr.items():
            c1, c2 = cats(j, ss[j] if sc_ == "s" else 1.0)
            catf[s] = (c1 if cat == 1 else c2).astype(np.float32)

        slmp = np.zeros((max(len(pairs), 1), 128, N), np.float32)
        for pi, (jA, jB) in enumerate(pairs):
            slmp[pi, 0:O] = ss[jA] * lm
            slmp[pi, O:128] = ss[jB] * lm
        slmb = np.zeros((max(len(solo_bf), 1), O, N), np.float32)
        for si, j in enumerate(solo_bf):
            slmb[si] = ss[j] * lm
        slmf = np.zeros((max(len(solo_f32), 1), O, N), np.float32)
        for si, j in enumerate(solo_f32):
            slmf[si] = ss[j] * lm

        in_maps.append({
            "catb": catb, "catf": catf, "ctT": ctT,
            "lmTb": lmT.astype(ml_dtypes.bfloat16), "lmTf": lmT,
            "lmp": lmp, "slmp": slmp, "slmb": slmb, "slmf": slmf,
            "eyeb": eyeb, "eyef": eyef,
        })

    global LAST
    res = run_bass_kernel_spmd(
        nc, in_maps, core_ids=list(range(NCORES)), trace=PROFILE
    )
    LAST = res
    out = np.zeros((O, O), dtype=np.float64)
    for c in range(NCORES):
        o = res.results[c]["out_e"].astype(np.float64)
        out += o[:, 0:O] + o[:, O:2 * O]
    return out.astype(np.float32)
